# revision 21
# baseline (speedup 1.0000x reference)
"""GQA kernel for Trainium2, 8 NeuronCores, group-per-core sharding.

Reference: B=2, S=2048, D=2048, H=32 heads, G=8 kv groups (GS=4, HD=64).
Core g owns kv group g (4 heads). Host pre-transposes x and weight slices
(cast to bf16) so every device matmul contracts over the partition axis;
host sums the 8 partial Wo projections (device output is bf16).

Device layout (bf16 operands, f32 psum):
  QH[h][b]  [128, S]  rows = [Q_h dims; Q_h dims] (duplicated)
  KT2[b]    [128, S]  rows = [K dims; K dims] (duplicated)
  vaug[b][kt] [128,65] v rows (natural) + ones col (softmax denominator)
  scores.T  [k=128, q<=512] psum = KT2tile.T @ QHslice = 2*(k.q); the
            duplication raises the matmul contraction from HD=64 to 128
            because bf16 matmuls stream at half rate below 128 rows
            (measured 427ns vs 216ns per 512-col matmul). The factor 2
            is folded into the exp scale (1/(8*2)).
  ctx.T     [65, 512] psum accum over k-tiles (row 64 = softmax sums)
  out       [t=128, o=512] psum = ctxn_pair.T @ woT_pair

Causality: only lower-triangular k-tiles are computed; the 4 diagonal
128-k-subtiles per q-chunk restrict the q-column range to [128*d, 512)
and apply one [128,128] triangular mask multiply on the first 128 cols.

Scheduling: one PSUM pool scope (4 rotating [128,512] banks shared by
projections/scores/rbc/out-proj + 4 dedicated ctx accumulator banks).
Batch-1 projections are emitted between batch-0 attention chunks; each
q-chunk's normalize + output projection is deferred into the next
q-chunk's k-loop so the PE never waits on the softmax-normalization
chain. ctx psum is evicted to SBUF (unnormalized bf16) right after
accumulation so the ctx banks turn over fast.
"""
import numpy as np
import ml_dtypes

import concourse.bacc as bacc
import concourse.mybir as mybir
import concourse.tile as tile
from concourse.bass_utils import run_bass_kernel_spmd

F32 = mybir.dt.float32
BF16 = mybir.dt.bfloat16
AF = mybir.ActivationFunctionType

B, S, D = 2, 2048, 2048
G, GS, HD = 8, 4, 64
T = B * S            # 4096 flattened tokens
QCH = 512            # q-chunk (psum free dim)
NQC = S // QCH       # 4 q-chunks per batch
NKT = S // 128       # 16 k-tiles per batch
NTC = T // QCH       # 8 proj T-chunks
NKD = D // 128       # 16 contraction tiles over D


def build_nc():
    nc = bacc.Bacc("TRN2", target_bir_lowering=False, debug=False)
    xT = nc.dram_tensor("xT", [D, T], BF16, kind="ExternalInput")
    wqT = nc.dram_tensor("wqT", [D, GS * HD], BF16, kind="ExternalInput")
    wkvT = nc.dram_tensor("wkvT", [D, 2 * HD], BF16, kind="ExternalInput")
    woT = nc.dram_tensor("woT", [GS * HD, D], BF16, kind="ExternalInput")
    masks = nc.dram_tensor("masks", [128, 128], BF16, kind="ExternalInput")
    aux = nc.dram_tensor("aux", [64, 64 + 4], BF16, kind="ExternalInput")
    sel = nc.dram_tensor("sel", [128, 256], BF16, kind="ExternalInput")
    outp = nc.dram_tensor("outp", [T, D], BF16, kind="ExternalOutput")

    with tile.TileContext(nc) as tc:
        with tc.tile_pool(name="const", bufs=1) as const, \
             tc.tile_pool(name="store", bufs=1) as store, \
             tc.tile_pool(name="xp", bufs=20) as xp, \
             tc.tile_pool(name="wp", bufs=16) as wp, \
             tc.tile_pool(name="cu", bufs=6) as cu, \
             tc.tile_pool(name="cn", bufs=4) as cn, \
             tc.tile_pool(name="ob", bufs=4) as ob, \
             tc.tile_pool(name="psMM", bufs=4, space="PSUM") as psMM, \
             tc.tile_pool(name="psC", bufs=1, space="PSUM") as psC:
            # --- static tiles -------------------------------------------------
            wq_sb = const.tile([128, NKD, GS * HD], BF16)
            nc.sync.dma_start(out=wq_sb[:], in_=xT_re(wqT, GS * HD))
            wkv_sb = const.tile([128, NKD, 2 * HD], BF16)
            nc.sync.dma_start(out=wkv_sb[:], in_=xT_re(wkvT, 2 * HD))
            wo_sb = [const.tile([128, D], BF16, tag=f"wo{p}", name=f"wo{p}") for p in range(2)]
            for p in range(2):
                nc.sync.dma_start(out=wo_sb[p][:], in_=woT[p * 128:(p + 1) * 128, :])
            mask_sb = const.tile([128, 128], BF16)
            aux_sb = const.tile([64, 64 + 4], BF16)
            sel_sb = const.tile([128, 256], BF16)
            nc.sync.dma_start(out=aux_sb[:], in_=aux[:])
            nc.sync.dma_start(out=mask_sb[:], in_=masks[:])
            nc.sync.dma_start(out=sel_sb[:], in_=sel[:])

            # long-lived activations (Q and K duplicated along partitions
            # so attention matmuls contract over 128 rows, not 64)
            QH = [[store.tile([128, S], BF16, tag=f"qh{h}{b}", name=f"qh{h}{b}")
                   for b in range(B)] for h in range(GS)]
            KT2 = [store.tile([128, S], BF16, tag=f"kt{b}", name=f"ktt{b}") for b in range(B)]
            VT = [store.tile([64, S], BF16, tag=f"vt{b}", name=f"vtt{b}") for b in range(B)]
            vaug = [[store.tile([128, HD + 1], BF16, tag=f"va{b}_{kt}", name=f"va{b}_{kt}")
                     for kt in range(NKT)] for b in range(B)]
            denq = store.tile([128, QCH], F32, tag="denq")
            nc.vector.memset(denq[:], 1.0)
            rrt = store.tile([128, QCH], BF16, tag="rrt")

            xre = xT.rearrange("(kt p) t -> p kt t", p=128)
            pending = []   # deferred normalize+out-proj closures

            def emit_proj_chunk(tch):
                b, col = tch // NQC, (tch % NQC) * QCH
                xt = []
                for kt in range(NKD):
                    xk = xp.tile([128, QCH], BF16, tag="xt", name=f"xt{kt}")
                    eng = nc.scalar if (tch < 4 and kt % 2 == 1) else nc.sync
                    eng.dma_start(
                        out=xk[:], in_=xre[:, kt, tch * QCH:(tch + 1) * QCH])
                    xt.append(xk)
                for p in range(2):
                    ps_q = psMM.tile([128, QCH], F32, tag="mm", name="ps_q")
                    for kt in range(NKD):
                        nc.tensor.matmul(
                            ps_q[:], wq_sb[:, kt, p * 128:(p + 1) * 128],
                            xt[kt][:], start=(kt == 0), stop=(kt == NKD - 1))
                    for half in range(2):
                        nc.vector.tensor_copy(
                            QH[2 * p][b][64 * half:64 * half + 64, col:col + QCH],
                            ps_q[0:64, :])
                        nc.vector.tensor_copy(
                            QH[2 * p + 1][b][64 * half:64 * half + 64, col:col + QCH],
                            ps_q[64:128, :])
                ps_kv = psMM.tile([128, QCH], F32, tag="mm", name="ps_kv")
                for kt in range(NKD):
                    nc.tensor.matmul(ps_kv[:], wkv_sb[:, kt, :], xt[kt][:],
                                     start=(kt == 0), stop=(kt == NKD - 1))
                for half in range(2):
                    nc.vector.tensor_copy(
                        KT2[b][64 * half:64 * half + 64, col:col + QCH],
                        ps_kv[0:64, :])
                nc.vector.tensor_copy(VT[b][:, col:col + QCH], ps_kv[64:128, :])

            def emit_vtrans(b):
                for kt in range(NKT):
                    ps_t = psMM.tile([128, HD], BF16, tag="mm", name="ps_t")
                    nc.tensor.transpose(
                        ps_t[:], VT[b][:, kt * 128:(kt + 1) * 128], aux_sb[0:64, 0:64])
                    nc.vector.tensor_copy(vaug[b][kt][:, 0:HD], ps_t[:])
                    nc.vector.memset(vaug[b][kt][:, HD:HD + 1], 1.0)

            def flush_pending():
                while pending:
                    pending.pop(0)()

            def emit_attn(b, qi):
                kmax = 4 * (qi + 1)
                ctx_ps = [psC.tile([HD + 1, QCH], F32, tag=f"ctx{h}", name=f"ctx{h}")
                          for h in range(GS)]

                def flush_ctx(item):
                    k0, ws = item
                    for h, (c0, w) in enumerate(ws):
                        nc.tensor.matmul(
                            ctx_ps[h][:, c0:QCH], vaug[b][k0][:], w[:, c0:QCH],
                            start=(k0 == 0), stop=(k0 == kmax - 1))

                pend = []  # 2-ktile software pipeline for MM2
                for kt in range(kmax):
                    dg = kt - 4 * qi
                    c0 = 128 * dg if dg >= 0 else 0
                    ws = []
                    for h in range(GS):
                        ps_s = psMM.tile([128, QCH], F32, tag="mm", name="ps_s")
                        qoff = qi * QCH
                        nc.tensor.matmul(
                            ps_s[:, c0:QCH],
                            KT2[b][:, kt * 128:(kt + 1) * 128],
                            QH[h][b][:, qoff + c0:qoff + QCH],
                            start=True, stop=True)
                        w = wp.tile([128, QCH], BF16, name="wt")
                        nc.scalar.activation(
                            w[:, c0:QCH], ps_s[:, c0:QCH], AF.Exp, scale=0.0625)
                        if dg >= 0:
                            nc.vector.tensor_mul(
                                w[:, c0:c0 + 128], w[:, c0:c0 + 128],
                                mask_sb[:, 0:128])
                        ws.append((c0, w))
                    pend.append((kt, ws))
                    if len(pend) > 3:
                        flush_ctx(pend.pop(0))
                    if kt >= 2 and pending:
                        pending.pop(0)()
                while pend:
                    flush_ctx(pend.pop(0))

                # evict unnormalized ctx to SBUF (frees the ctx banks) and
                # gather denominator rows (32-aligned for ACT)
                ctxu = [cu.tile([128, QCH], BF16, tag=f"cu{p}", name=f"cu{p}")
                        for p in range(2)]
                for h in range(GS):
                    p, hb = h // 2, (h % 2) * 64
                    if h % 2 == 0:
                        nc.vector.tensor_copy(ctxu[p][hb:hb + 64, :], ctx_ps[h][0:64, :])
                    else:
                        nc.scalar.activation(ctxu[p][hb:hb + 64, :], ctx_ps[h][0:64, :], AF.Copy)
                    nc.scalar.activation(
                        denq[32 * h:32 * h + 1, :], ctx_ps[h][64:65, :], AF.Copy)
                with nc.allow_low_precision(reason="softmax recip bf16"):
                    nc.vector.reciprocal(rrt[:, 0:QCH // 2], denq[:, 0:QCH // 2])
                    nc.vector.reciprocal(rrt[:, QCH // 2:QCH], denq[:, QCH // 2:QCH])

                ctxn_cell = []

                def tail0(b=b, qi=qi, ctxu=ctxu, ctxn_cell=ctxn_cell):
                    ctxn = [cn.tile([128, QCH], BF16, tag=f"cn{p}", name=f"cn{p}")
                            for p in range(2)]
                    ctxn_cell.extend(ctxn)
                    H = QCH // 2
                    for hf in range(2):
                        for p in range(2):
                            ps_r = psMM.tile([128, H], F32, tag="mm", name="ps_r")
                            nc.tensor.matmul(
                                ps_r[:], sel_sb[:, p * 128:(p + 1) * 128],
                                rrt[:, hf * H:(hf + 1) * H], start=True, stop=True)
                            nc.vector.tensor_mul(
                                ctxn[p][:, hf * H:(hf + 1) * H],
                                ctxu[p][:, hf * H:(hf + 1) * H], ps_r[:])
                pending.append(tail0)

                def make_tt(tt, b=b, qi=qi, ctxn_cell=ctxn_cell):
                    def piece():
                        ctxn = ctxn_cell
                        osb = ob.tile([128, D], BF16)
                        for oc in range(D // 512):
                            ps_o = psMM.tile([128, 512], F32, tag="mm", name="ps_o")
                            for p in range(2):
                                nc.tensor.matmul(
                                    ps_o[:], ctxn[p][:, tt * 128:(tt + 1) * 128],
                                    wo_sb[p][:, oc * 512:(oc + 1) * 512],
                                    start=(p == 0), stop=(p == 1))
                            if oc % 2 == 0:
                                nc.vector.tensor_copy(
                                    osb[:, oc * 512:(oc + 1) * 512], ps_o[:])
                            else:
                                nc.scalar.activation(
                                    osb[:, oc * 512:(oc + 1) * 512], ps_o[:], AF.Copy)
                        row = b * S + qi * QCH + tt * 128
                        nc.sync.dma_start(out=outp[row:row + 128, :], in_=osb[:])
                    return piece
                for tt in range(QCH // 128):
                    pending.append(make_tt(tt))

            # --- emission schedule -------------------------------------------
            for tch in range(4):
                emit_proj_chunk(tch)
            emit_vtrans(0)
            for qi in range(NQC):
                emit_attn(0, qi)
                emit_proj_chunk(4 + qi)
                if qi == NQC - 1:
                    emit_vtrans(1)
            for qi in range(NQC):
                emit_attn(1, qi)
            flush_pending()
    nc.compile()
    return nc


def xT_re(t, c):
    return t.rearrange("(kt p) c -> p kt c", p=128)


def prep_inputs(x, Wq, Wk, Wv, Wo):
    bf = ml_dtypes.bfloat16
    xT = np.ascontiguousarray(x.reshape(T, D).T).astype(bf)
    km = np.arange(128)[:, None]
    qm = np.arange(128)[None, :]
    masks = (km <= qm).astype(bf)                     # [128,128] causal keep
    aux = np.zeros((64, 68), dtype=bf)
    aux[:64, :64] = np.eye(64, dtype=bf)
    sel = np.zeros((128, 256), dtype=np.float32)      # rbc head selection
    for p in range(2):
        for j in range(128):
            sel[32 * (2 * p + j // 64), p * 128 + j] = 1.0
    sel = sel.astype(bf)
    in_maps = []
    for g in range(G):
        in_maps.append({
            "xT": xT,
            "wqT": np.ascontiguousarray(Wq[g * GS * HD:(g + 1) * GS * HD, :].T).astype(bf),
            "wkvT": np.ascontiguousarray(
                np.concatenate([Wk[g * HD:(g + 1) * HD, :],
                                Wv[g * HD:(g + 1) * HD, :]], axis=0).T).astype(bf),
            "woT": np.ascontiguousarray(Wo[:, g * GS * HD:(g + 1) * GS * HD].T).astype(bf),
            "masks": masks,
            "aux": aux,
            "sel": sel,
        })
    return in_maps


def kernel(x, Wq, Wk, Wv, Wo):
    x = np.asarray(x, dtype=np.float32)
    in_maps = prep_inputs(np.asarray(x, np.float32), np.asarray(Wq, np.float32),
                          np.asarray(Wk, np.float32), np.asarray(Wv, np.float32),
                          np.asarray(Wo, np.float32))
    nc = build_nc()
    res = run_bass_kernel_spmd(nc, in_maps, list(range(G)))
    acc = np.zeros((T, D), dtype=np.float64)
    for g in range(G):
        acc += res.results[g]["outp"].astype(np.float64)
    return acc.astype(np.float32).reshape(B, S, D)


# revision 23
# speedup vs baseline: 1.0083x; 1.0083x over previous
"""GQA kernel for Trainium2, 8 NeuronCores, group-per-core sharding.

Reference: B=2, S=2048, D=2048, H=32 heads, G=8 kv groups (GS=4, HD=64).
Core g owns kv group g (4 heads). Host pre-transposes x and weight slices
(cast to bf16) so every device matmul contracts over the partition axis;
host sums the 8 partial Wo projections (device output is bf16).

Device layout (bf16 operands, f32 psum):
  QH[h][b]  [128, S]  rows = [Q_h dims; Q_h dims] (duplicated)
  KT2[b]    [128, S]  rows = [K dims; K dims] (duplicated)
  vaug[b][kt] [128,65] v rows (natural) + ones col (softmax denominator)
  scores.T  [k=128, q<=512] psum = KT2tile.T @ QHslice = 2*(k.q); the
            duplication raises the matmul contraction from HD=64 to 128
            because bf16 matmuls stream at half rate below 128 rows
            (measured 427ns vs 216ns per 512-col matmul). The factor 2
            is folded into the exp scale (1/(8*2)).
  ctx.T     [65, 512] psum accum over k-tiles (row 64 = softmax sums)
  out       [t=128, o=512] psum = ctxn_pair.T @ woT_pair

Causality: only lower-triangular k-tiles are computed; the 4 diagonal
128-k-subtiles per q-chunk restrict the q-column range to [128*d, 512)
and apply one [128,128] triangular mask multiply on the first 128 cols.

Scheduling: one PSUM pool scope (4 rotating [128,512] banks shared by
projections/scores/rbc/out-proj + 4 dedicated ctx accumulator banks).
Batch-1 projections are emitted between batch-0 attention chunks; each
q-chunk's normalize + output projection is deferred into the next
q-chunk's k-loop so the PE never waits on the softmax-normalization
chain. ctx psum is evicted to SBUF (unnormalized bf16) right after
accumulation so the ctx banks turn over fast.
"""
import numpy as np
import ml_dtypes

import concourse.bacc as bacc
import concourse.mybir as mybir
import concourse.tile as tile
from concourse.bass_utils import run_bass_kernel_spmd

F32 = mybir.dt.float32
BF16 = mybir.dt.bfloat16
AF = mybir.ActivationFunctionType

B, S, D = 2, 2048, 2048
G, GS, HD = 8, 4, 64
T = B * S            # 4096 flattened tokens
QCH = 512            # q-chunk (psum free dim)
NQC = S // QCH       # 4 q-chunks per batch
NKT = S // 128       # 16 k-tiles per batch
NTC = T // QCH       # 8 proj T-chunks
NKD = D // 128       # 16 contraction tiles over D


def build_nc():
    nc = bacc.Bacc("TRN2", target_bir_lowering=False, debug=False)
    xT = nc.dram_tensor("xT", [D, T], BF16, kind="ExternalInput")
    wqT = nc.dram_tensor("wqT", [D, GS * HD], BF16, kind="ExternalInput")
    wkvT = nc.dram_tensor("wkvT", [D, 2 * HD], BF16, kind="ExternalInput")
    woT = nc.dram_tensor("woT", [GS * HD, D], BF16, kind="ExternalInput")
    masks = nc.dram_tensor("masks", [128, 128], BF16, kind="ExternalInput")
    aux = nc.dram_tensor("aux", [64, 64 + 4], BF16, kind="ExternalInput")
    sel = nc.dram_tensor("sel", [128, 256], BF16, kind="ExternalInput")
    outp = nc.dram_tensor("outp", [T, D], BF16, kind="ExternalOutput")

    with tile.TileContext(nc) as tc:
        with tc.tile_pool(name="const", bufs=1) as const, \
             tc.tile_pool(name="store", bufs=1) as store, \
             tc.tile_pool(name="xp", bufs=20) as xp, \
             tc.tile_pool(name="wp", bufs=16) as wp, \
             tc.tile_pool(name="cu", bufs=6) as cu, \
             tc.tile_pool(name="cn", bufs=4) as cn, \
             tc.tile_pool(name="ob", bufs=4) as ob, \
             tc.tile_pool(name="psMM", bufs=4, space="PSUM") as psMM, \
             tc.tile_pool(name="psC", bufs=1, space="PSUM") as psC:
            # --- static tiles -------------------------------------------------
            wq_sb = const.tile([128, NKD, GS * HD], BF16)
            nc.sync.dma_start(out=wq_sb[:], in_=xT_re(wqT, GS * HD))
            wkv_sb = const.tile([128, NKD, 2 * HD], BF16)
            nc.sync.dma_start(out=wkv_sb[:], in_=xT_re(wkvT, 2 * HD))
            wo_sb = [const.tile([128, D], BF16, tag=f"wo{p}", name=f"wo{p}") for p in range(2)]
            for p in range(2):
                nc.sync.dma_start(out=wo_sb[p][:], in_=woT[p * 128:(p + 1) * 128, :])
            mask_sb = const.tile([128, 128], BF16)
            aux_sb = const.tile([64, 64 + 4], BF16)
            sel_sb = const.tile([128, 256], BF16)
            nc.sync.dma_start(out=aux_sb[:], in_=aux[:])
            nc.sync.dma_start(out=mask_sb[:], in_=masks[:])
            nc.sync.dma_start(out=sel_sb[:], in_=sel[:])

            # long-lived activations (Q and K duplicated along partitions
            # so attention matmuls contract over 128 rows, not 64)
            QH = [[store.tile([128, S], BF16, tag=f"qh{h}{b}", name=f"qh{h}{b}")
                   for b in range(B)] for h in range(GS)]
            KT2 = [store.tile([128, S], BF16, tag=f"kt{b}", name=f"ktt{b}") for b in range(B)]
            VT = [store.tile([64, S], BF16, tag=f"vt{b}", name=f"vtt{b}") for b in range(B)]
            vaug = [[store.tile([128, HD + 1], BF16, tag=f"va{b}_{kt}", name=f"va{b}_{kt}")
                     for kt in range(NKT)] for b in range(B)]
            denq = store.tile([128, QCH], F32, tag="denq")
            nc.vector.memset(denq[:], 1.0)
            rrt = store.tile([128, QCH], BF16, tag="rrt")

            xre = xT.rearrange("(kt p) t -> p kt t", p=128)
            pending = []   # deferred normalize+out-proj closures

            def emit_xt_loads(tch):
                xt = []
                for kt in range(NKD):
                    xk = xp.tile([128, QCH], BF16, tag="xt", name=f"xt{kt}")
                    eng = nc.scalar if (tch == 0 and kt % 2 == 1) else nc.sync
                    eng.dma_start(
                        out=xk[:], in_=xre[:, kt, tch * QCH:(tch + 1) * QCH])
                    xt.append(xk)
                return xt

            def emit_proj_chunk(tch, xt):
                b, col = tch // NQC, (tch % NQC) * QCH
                for p in range(2):
                    ps_q = psMM.tile([128, QCH], F32, tag="mm", name="ps_q")
                    for kt in range(NKD):
                        nc.tensor.matmul(
                            ps_q[:], wq_sb[:, kt, p * 128:(p + 1) * 128],
                            xt[kt][:], start=(kt == 0), stop=(kt == NKD - 1))
                    for half in range(2):
                        nc.vector.tensor_copy(
                            QH[2 * p][b][64 * half:64 * half + 64, col:col + QCH],
                            ps_q[0:64, :])
                        nc.vector.tensor_copy(
                            QH[2 * p + 1][b][64 * half:64 * half + 64, col:col + QCH],
                            ps_q[64:128, :])
                ps_kv = psMM.tile([128, QCH], F32, tag="mm", name="ps_kv")
                for kt in range(NKD):
                    nc.tensor.matmul(ps_kv[:], wkv_sb[:, kt, :], xt[kt][:],
                                     start=(kt == 0), stop=(kt == NKD - 1))
                for half in range(2):
                    nc.vector.tensor_copy(
                        KT2[b][64 * half:64 * half + 64, col:col + QCH],
                        ps_kv[0:64, :])
                nc.vector.tensor_copy(VT[b][:, col:col + QCH], ps_kv[64:128, :])

            def emit_vtrans(b, kts=None):
                for kt in (range(NKT) if kts is None else kts):
                    ps_t = psMM.tile([128, HD], BF16, tag="mm", name="ps_t")
                    nc.tensor.transpose(
                        ps_t[:], VT[b][:, kt * 128:(kt + 1) * 128], aux_sb[0:64, 0:64])
                    nc.vector.tensor_copy(vaug[b][kt][:, 0:HD], ps_t[:])
                    nc.vector.memset(vaug[b][kt][:, HD:HD + 1], 1.0)

            def flush_pending():
                while pending:
                    pending.pop(0)()

            def emit_attn(b, qi):
                kmax = 4 * (qi + 1)
                ctx_ps = [psC.tile([HD + 1, QCH], F32, tag=f"ctx{h}", name=f"ctx{h}")
                          for h in range(GS)]

                def flush_ctx(item):
                    k0, ws = item
                    for h, (c0, w) in enumerate(ws):
                        nc.tensor.matmul(
                            ctx_ps[h][:, c0:QCH], vaug[b][k0][:], w[:, c0:QCH],
                            start=(k0 == 0), stop=(k0 == kmax - 1))

                pend = []  # 2-ktile software pipeline for MM2
                for kt in range(kmax):
                    dg = kt - 4 * qi
                    c0 = 128 * dg if dg >= 0 else 0
                    ws = []
                    for h in range(GS):
                        ps_s = psMM.tile([128, QCH], F32, tag="mm", name="ps_s")
                        qoff = qi * QCH
                        nc.tensor.matmul(
                            ps_s[:, c0:QCH],
                            KT2[b][:, kt * 128:(kt + 1) * 128],
                            QH[h][b][:, qoff + c0:qoff + QCH],
                            start=True, stop=True)
                        w = wp.tile([128, QCH], BF16, name="wt")
                        nc.scalar.activation(
                            w[:, c0:QCH], ps_s[:, c0:QCH], AF.Exp, scale=0.0625)
                        if dg >= 0:
                            nc.vector.tensor_mul(
                                w[:, c0:c0 + 128], w[:, c0:c0 + 128],
                                mask_sb[:, 0:128])
                        ws.append((c0, w))
                    pend.append((kt, ws))
                    if len(pend) > 3:
                        flush_ctx(pend.pop(0))
                    if kt >= 2 and pending:
                        pending.pop(0)()
                while pend:
                    flush_ctx(pend.pop(0))

                # evict unnormalized ctx to SBUF (frees the ctx banks) and
                # gather denominator rows (32-aligned for ACT)
                ctxu = [cu.tile([128, QCH], BF16, tag=f"cu{p}", name=f"cu{p}")
                        for p in range(2)]
                for h in range(GS):
                    nc.scalar.activation(
                        denq[32 * h:32 * h + 1, :], ctx_ps[h][64:65, :], AF.Copy)
                for h in range(GS):
                    p, hb = h // 2, (h % 2) * 64
                    if h % 2 == 0:
                        nc.vector.tensor_copy(ctxu[p][hb:hb + 64, :], ctx_ps[h][0:64, :])
                    else:
                        nc.scalar.activation(ctxu[p][hb:hb + 64, :], ctx_ps[h][0:64, :], AF.Copy)
                with nc.allow_low_precision(reason="softmax recip bf16"):
                    nc.vector.reciprocal(rrt[:, 0:QCH // 2], denq[:, 0:QCH // 2])
                    nc.vector.reciprocal(rrt[:, QCH // 2:QCH], denq[:, QCH // 2:QCH])

                ctxn_cell = []

                def tail0(b=b, qi=qi, ctxu=ctxu, ctxn_cell=ctxn_cell):
                    ctxn = [cn.tile([128, QCH], BF16, tag=f"cn{p}", name=f"cn{p}")
                            for p in range(2)]
                    ctxn_cell.extend(ctxn)
                    H = QCH // 2
                    for hf in range(2):
                        for p in range(2):
                            ps_r = psMM.tile([128, H], F32, tag="mm", name="ps_r")
                            nc.tensor.matmul(
                                ps_r[:], sel_sb[:, p * 128:(p + 1) * 128],
                                rrt[:, hf * H:(hf + 1) * H], start=True, stop=True)
                            nc.vector.tensor_mul(
                                ctxn[p][:, hf * H:(hf + 1) * H],
                                ctxu[p][:, hf * H:(hf + 1) * H], ps_r[:])
                pending.append(tail0)

                def make_tt(tt, b=b, qi=qi, ctxn_cell=ctxn_cell):
                    def piece():
                        ctxn = ctxn_cell
                        osb = ob.tile([128, D], BF16)
                        for oc in range(D // 512):
                            ps_o = psMM.tile([128, 512], F32, tag="mm", name="ps_o")
                            for p in range(2):
                                nc.tensor.matmul(
                                    ps_o[:], ctxn[p][:, tt * 128:(tt + 1) * 128],
                                    wo_sb[p][:, oc * 512:(oc + 1) * 512],
                                    start=(p == 0), stop=(p == 1))
                            if oc % 2 == 0:
                                nc.vector.tensor_copy(
                                    osb[:, oc * 512:(oc + 1) * 512], ps_o[:])
                            else:
                                nc.scalar.activation(
                                    osb[:, oc * 512:(oc + 1) * 512], ps_o[:], AF.Copy)
                        row = b * S + qi * QCH + tt * 128
                        nc.sync.dma_start(out=outp[row:row + 128, :], in_=osb[:])
                    return piece
                for tt in range(QCH // 128):
                    pending.append(make_tt(tt))

            # --- emission schedule: proj chunk -> vtrans slice -> attn chunk,
            # with the next chunk's x-tiles prefetched BEFORE the attention
            # tails queue their output stores on the same DMA queue
            xt_cur = emit_xt_loads(0)
            for b in range(B):
                for qi in range(NQC):
                    tch = 4 * b + qi
                    emit_proj_chunk(tch, xt_cur)
                    emit_vtrans(b, range(4 * qi, 4 * qi + 4))
                    if tch + 1 < NTC:
                        xt_next = emit_xt_loads(tch + 1)
                    emit_attn(b, qi)
                    xt_cur = xt_next
            flush_pending()
    nc.compile()
    return nc


def xT_re(t, c):
    return t.rearrange("(kt p) c -> p kt c", p=128)


def prep_inputs(x, Wq, Wk, Wv, Wo):
    bf = ml_dtypes.bfloat16
    xT = np.ascontiguousarray(x.reshape(T, D).T).astype(bf)
    km = np.arange(128)[:, None]
    qm = np.arange(128)[None, :]
    masks = (km <= qm).astype(bf)                     # [128,128] causal keep
    aux = np.zeros((64, 68), dtype=bf)
    aux[:64, :64] = np.eye(64, dtype=bf)
    sel = np.zeros((128, 256), dtype=np.float32)      # rbc head selection
    for p in range(2):
        for j in range(128):
            sel[32 * (2 * p + j // 64), p * 128 + j] = 1.0
    sel = sel.astype(bf)
    in_maps = []
    for g in range(G):
        in_maps.append({
            "xT": xT,
            "wqT": np.ascontiguousarray(Wq[g * GS * HD:(g + 1) * GS * HD, :].T).astype(bf),
            "wkvT": np.ascontiguousarray(
                np.concatenate([Wk[g * HD:(g + 1) * HD, :],
                                Wv[g * HD:(g + 1) * HD, :]], axis=0).T).astype(bf),
            "woT": np.ascontiguousarray(Wo[:, g * GS * HD:(g + 1) * GS * HD].T).astype(bf),
            "masks": masks,
            "aux": aux,
            "sel": sel,
        })
    return in_maps


def kernel(x, Wq, Wk, Wv, Wo):
    x = np.asarray(x, dtype=np.float32)
    in_maps = prep_inputs(np.asarray(x, np.float32), np.asarray(Wq, np.float32),
                          np.asarray(Wk, np.float32), np.asarray(Wv, np.float32),
                          np.asarray(Wo, np.float32))
    nc = build_nc()
    res = run_bass_kernel_spmd(nc, in_maps, list(range(G)))
    acc = np.zeros((T, D), dtype=np.float64)
    for g in range(G):
        acc += res.results[g]["outp"].astype(np.float64)
    return acc.astype(np.float32).reshape(B, S, D)


# revision 29
# speedup vs baseline: 1.0761x; 1.0672x over previous
"""GQA kernel for Trainium2, 8 NeuronCores, group-per-core sharding.

Reference: B=2, S=2048, D=2048, H=32 heads, G=8 kv groups (GS=4, HD=64).
Core g owns kv group g (4 heads). Host pre-transposes x and weight slices
(cast to bf16) so every device matmul contracts over the partition axis;
host sums the 8 partial Wo projections (device output is bf16).

Device layout (bf16 operands, f32 psum):
  QH[h][b]  [128, S]  rows = [Q_h dims; Q_h dims] (duplicated)
  KT2[b]    [128, S]  rows = [K dims; K dims] (duplicated)
  vaug[b][kt] [128,65] v rows (natural) + ones col (softmax denominator)
  scores.T  [k=128, q<=512] psum = KT2tile.T @ QHslice = 2*(k.q); the
            duplication raises the matmul contraction from HD=64 to 128
            because bf16 matmuls stream at half rate below 128 rows
            (measured 427ns vs 216ns per 512-col matmul). The factor 2
            is folded into the exp scale (1/(8*2)).
  ctx.T     [65, 512] psum accum over k-tiles (row 64 = softmax sums)
  out       [t=128, o=512] psum = ctxn_pair.T @ woT_pair

Causality: only lower-triangular k-tiles are computed; the 4 diagonal
128-k-subtiles per q-chunk restrict the q-column range to [128*d, 512)
and apply one [128,128] triangular mask multiply on the first 128 cols.

Scheduling: one PSUM pool scope (4 rotating [128,512] banks shared by
projections/scores/rbc/out-proj + 4 dedicated ctx accumulator banks).
Batch-1 projections are emitted between batch-0 attention chunks; each
q-chunk's normalize + output projection is deferred into the next
q-chunk's k-loop so the PE never waits on the softmax-normalization
chain. ctx psum is evicted to SBUF (unnormalized bf16) right after
accumulation so the ctx banks turn over fast.
"""
import numpy as np
import ml_dtypes

import concourse.bacc as bacc
import concourse.mybir as mybir
import concourse.tile as tile
from concourse.bass_utils import run_bass_kernel_spmd

F32 = mybir.dt.float32
BF16 = mybir.dt.bfloat16
AF = mybir.ActivationFunctionType

B, S, D = 2, 2048, 2048
G, GS, HD = 8, 4, 64
T = B * S            # 4096 flattened tokens
QCH = 512            # q-chunk (psum free dim)
NQC = S // QCH       # 4 q-chunks per batch
NKT = S // 128       # 16 k-tiles per batch
NTC = T // QCH       # 8 proj T-chunks
NKD = D // 128       # 16 contraction tiles over D


def build_nc():
    nc = bacc.Bacc("TRN2", target_bir_lowering=False, debug=False)
    xT = nc.dram_tensor("xT", [D, T], BF16, kind="ExternalInput")
    wqT = nc.dram_tensor("wqT", [D, GS * HD], BF16, kind="ExternalInput")
    wkvT = nc.dram_tensor("wkvT", [D, 2 * HD], BF16, kind="ExternalInput")
    woT = nc.dram_tensor("woT", [GS * HD, D], BF16, kind="ExternalInput")
    masks = nc.dram_tensor("masks", [128, 128], BF16, kind="ExternalInput")
    aux = nc.dram_tensor("aux", [64, 64 + 4], BF16, kind="ExternalInput")
    sel = nc.dram_tensor("sel", [128, 256], BF16, kind="ExternalInput")
    outp = nc.dram_tensor("outp", [T, D], BF16, kind="ExternalOutput")

    with tile.TileContext(nc) as tc:
        with tc.tile_pool(name="const", bufs=1) as const, \
             tc.tile_pool(name="store", bufs=1) as store, \
             tc.tile_pool(name="xp", bufs=20) as xp, \
             tc.tile_pool(name="wp", bufs=16) as wp, \
             tc.tile_pool(name="cu", bufs=6) as cu, \
             tc.tile_pool(name="cn", bufs=4) as cn, \
             tc.tile_pool(name="ob", bufs=4) as ob, \
             tc.tile_pool(name="psMM", bufs=4, space="PSUM") as psMM, \
             tc.tile_pool(name="psC", bufs=1, space="PSUM") as psC:
            # --- static tiles: wq sliced per-kt on the ACT queue ahead of
            # the odd x-tiles; everything not needed immediately deferred
            wq_sb = const.tile([128, NKD, GS * HD], BF16)
            wqre = xT_re(wqT, GS * HD)
            for kt in range(NKD):
                nc.scalar.dma_start(out=wq_sb[:, kt, :], in_=wqre[:, kt, :])
            wkv_sb = const.tile([128, NKD, 2 * HD], BF16)
            wo_sb = [const.tile([128, D], BF16, tag=f"wo{p}", name=f"wo{p}") for p in range(2)]
            mask_sb = const.tile([128, 128], BF16)
            aux_sb = const.tile([64, 64 + 4], BF16)
            sel_sb = const.tile([128, 256], BF16)

            def emit_deferred_consts():
                nc.sync.dma_start(out=wkv_sb[:], in_=xT_re(wkvT, 2 * HD))
                for p in range(2):
                    nc.sync.dma_start(out=wo_sb[p][:], in_=woT[p * 128:(p + 1) * 128, :])
                nc.scalar.dma_start(out=aux_sb[:], in_=aux[:])
                nc.scalar.dma_start(out=mask_sb[:], in_=masks[:])
                nc.scalar.dma_start(out=sel_sb[:], in_=sel[:])

            # long-lived activations (Q and K duplicated along partitions
            # so attention matmuls contract over 128 rows, not 64)
            QH = [[store.tile([128, S], BF16, tag=f"qh{h}{b}", name=f"qh{h}{b}")
                   for b in range(B)] for h in range(GS)]
            KT2 = [store.tile([128, S], BF16, tag=f"kt{b}", name=f"ktt{b}") for b in range(B)]
            VT = [store.tile([64, S], BF16, tag=f"vt{b}", name=f"vtt{b}") for b in range(B)]
            vaug = [[store.tile([128, HD + 1], BF16, tag=f"va{b}_{kt}", name=f"va{b}_{kt}")
                     for kt in range(NKT)] for b in range(B)]
            denq = store.tile([128, QCH], F32, tag="denq")
            nc.vector.memset(denq[:], 1.0)
            rrt = store.tile([128, QCH], BF16, tag="rrt")

            xre = xT.rearrange("(kt p) t -> p kt t", p=128)
            pending = []   # deferred normalize+out-proj closures

            def emit_xt_loads(tch):
                xt = []
                for kt in range(NKD):
                    xk = xp.tile([128, QCH], BF16, tag="xt", name=f"xt{kt}")
                    eng = nc.scalar if (tch == 0 and kt % 2 == 1) else nc.sync
                    eng.dma_start(
                        out=xk[:], in_=xre[:, kt, tch * QCH:(tch + 1) * QCH])
                    xt.append(xk)
                return xt

            def emit_proj_chunk(tch, xt):
                b, col = tch // NQC, (tch % NQC) * QCH
                # KV first (V gates the transposes, K gates every score),
                # then Q per head, copies split across DVE and ACT so the
                # PE never waits ~8 serialized copies at the chunk boundary
                ps_kv = psMM.tile([128, QCH], F32, tag="mm", name="ps_kv")
                for kt in range(NKD):
                    nc.tensor.matmul(ps_kv[:], wkv_sb[:, kt, :], xt[kt][:],
                                     start=(kt == 0), stop=(kt == NKD - 1))
                nc.vector.tensor_copy(VT[b][:, col:col + QCH], ps_kv[64:128, :])
                for half in range(2):
                    nc.scalar.activation(
                        KT2[b][64 * half:64 * half + 64, col:col + QCH],
                        ps_kv[0:64, :], AF.Copy)
                for p in range(2):
                    ps_q = psMM.tile([128, QCH], F32, tag="mm", name="ps_q")
                    for kt in range(NKD):
                        nc.tensor.matmul(
                            ps_q[:], wq_sb[:, kt, p * 128:(p + 1) * 128],
                            xt[kt][:], start=(kt == 0), stop=(kt == NKD - 1))
                    for half in range(2):
                        nc.vector.tensor_copy(
                            QH[2 * p][b][64 * half:64 * half + 64, col:col + QCH],
                            ps_q[0:64, :])
                        nc.scalar.activation(
                            QH[2 * p + 1][b][64 * half:64 * half + 64, col:col + QCH],
                            ps_q[64:128, :], AF.Copy)

            def emit_vtrans(b, kts=None):
                for kt in (range(NKT) if kts is None else kts):
                    ps_t = psMM.tile([128, HD], BF16, tag="mm", name="ps_t")
                    nc.tensor.transpose(
                        ps_t[:], VT[b][:, kt * 128:(kt + 1) * 128], aux_sb[0:64, 0:64])
                    nc.vector.tensor_copy(vaug[b][kt][:, 0:HD], ps_t[:])
                    nc.vector.memset(vaug[b][kt][:, HD:HD + 1], 1.0)

            def flush_pending():
                while pending:
                    pending.pop(0)()

            def emit_attn(b, qi):
                kmax = 4 * (qi + 1)
                ctx_ps = [psC.tile([HD + 1, QCH], F32, tag=f"ctx{h}", name=f"ctx{h}")
                          for h in range(GS)]

                def flush_ctx(item):
                    k0, ws = item
                    for h, (c0, w) in enumerate(ws):
                        nc.tensor.matmul(
                            ctx_ps[h][:, c0:QCH], vaug[b][k0][:], w[:, c0:QCH],
                            start=(k0 == 0), stop=(k0 == kmax - 1))

                pend = []  # 2-ktile software pipeline for MM2
                for kt in range(kmax):
                    dg = kt - 4 * qi
                    c0 = 128 * dg if dg >= 0 else 0
                    ws = []
                    for h in range(GS):
                        ps_s = psMM.tile([128, QCH], F32, tag="mm", name="ps_s")
                        qoff = qi * QCH
                        nc.tensor.matmul(
                            ps_s[:, c0:QCH],
                            KT2[b][:, kt * 128:(kt + 1) * 128],
                            QH[h][b][:, qoff + c0:qoff + QCH],
                            start=True, stop=True)
                        w = wp.tile([128, QCH], BF16, name="wt")
                        nc.scalar.activation(
                            w[:, c0:QCH], ps_s[:, c0:QCH], AF.Exp, scale=0.0625)
                        if dg >= 0:
                            nc.vector.tensor_mul(
                                w[:, c0:c0 + 128], w[:, c0:c0 + 128],
                                mask_sb[:, 0:128])
                        ws.append((c0, w))
                    pend.append((kt, ws))
                    if len(pend) > 3:
                        flush_ctx(pend.pop(0))
                    if kt >= 2 and pending:
                        pending.pop(0)()
                while pend:
                    flush_ctx(pend.pop(0))

                # evict unnormalized ctx to SBUF (frees the ctx banks) and
                # gather denominator rows (32-aligned for ACT)
                ctxu = [cu.tile([128, QCH], BF16, tag=f"cu{p}", name=f"cu{p}")
                        for p in range(2)]
                for h in range(GS):
                    nc.scalar.activation(
                        denq[32 * h:32 * h + 1, :], ctx_ps[h][64:65, :], AF.Copy)
                for h in range(GS):
                    p, hb = h // 2, (h % 2) * 64
                    if h % 2 == 0:
                        nc.vector.tensor_copy(ctxu[p][hb:hb + 64, :], ctx_ps[h][0:64, :])
                    else:
                        nc.scalar.activation(ctxu[p][hb:hb + 64, :], ctx_ps[h][0:64, :], AF.Copy)
                with nc.allow_low_precision(reason="softmax recip bf16"):
                    nc.vector.reciprocal(rrt[:, 0:QCH // 2], denq[:, 0:QCH // 2])
                    nc.vector.reciprocal(rrt[:, QCH // 2:QCH], denq[:, QCH // 2:QCH])

                ctxn_cell = []

                def tail0(b=b, qi=qi, ctxu=ctxu, ctxn_cell=ctxn_cell):
                    ctxn = [cn.tile([128, QCH], BF16, tag=f"cn{p}", name=f"cn{p}")
                            for p in range(2)]
                    ctxn_cell.extend(ctxn)
                    H = QCH // 2
                    for hf in range(2):
                        for p in range(2):
                            ps_r = psMM.tile([128, H], F32, tag="mm", name="ps_r")
                            nc.tensor.matmul(
                                ps_r[:], sel_sb[:, p * 128:(p + 1) * 128],
                                rrt[:, hf * H:(hf + 1) * H], start=True, stop=True)
                            nc.vector.tensor_mul(
                                ctxn[p][:, hf * H:(hf + 1) * H],
                                ctxu[p][:, hf * H:(hf + 1) * H], ps_r[:])
                pending.append(tail0)

                def make_tt(tt, b=b, qi=qi, ctxn_cell=ctxn_cell):
                    def piece():
                        ctxn = ctxn_cell
                        osb = ob.tile([128, D], BF16)
                        for oc in range(D // 512):
                            ps_o = psMM.tile([128, 512], F32, tag="mm", name="ps_o")
                            for p in range(2):
                                nc.tensor.matmul(
                                    ps_o[:], ctxn[p][:, tt * 128:(tt + 1) * 128],
                                    wo_sb[p][:, oc * 512:(oc + 1) * 512],
                                    start=(p == 0), stop=(p == 1))
                            if oc % 2 == 0:
                                nc.vector.tensor_copy(
                                    osb[:, oc * 512:(oc + 1) * 512], ps_o[:])
                            else:
                                nc.scalar.activation(
                                    osb[:, oc * 512:(oc + 1) * 512], ps_o[:], AF.Copy)
                        row = b * S + qi * QCH + tt * 128
                        nc.sync.dma_start(out=outp[row:row + 128, :], in_=osb[:])
                    return piece
                for tt in range(QCH // 128):
                    pending.append(make_tt(tt))

            # --- emission schedule: proj chunk -> vtrans slice -> attn chunk,
            # with the next chunk's x-tiles prefetched BEFORE the attention
            # tails queue their output stores on the same DMA queue
            xt_cur = emit_xt_loads(0)
            emit_deferred_consts()
            for b in range(B):
                for qi in range(NQC):
                    tch = 4 * b + qi
                    emit_proj_chunk(tch, xt_cur)
                    emit_vtrans(b, range(4 * qi, 4 * qi + 4))
                    if tch + 1 < NTC:
                        xt_next = emit_xt_loads(tch + 1)
                    emit_attn(b, qi)
                    xt_cur = xt_next
            flush_pending()
    nc.compile()
    return nc


def xT_re(t, c):
    return t.rearrange("(kt p) c -> p kt c", p=128)


def prep_inputs(x, Wq, Wk, Wv, Wo):
    bf = ml_dtypes.bfloat16
    xT = np.ascontiguousarray(x.reshape(T, D).T).astype(bf)
    km = np.arange(128)[:, None]
    qm = np.arange(128)[None, :]
    masks = (km <= qm).astype(bf)                     # [128,128] causal keep
    aux = np.zeros((64, 68), dtype=bf)
    aux[:64, :64] = np.eye(64, dtype=bf)
    sel = np.zeros((128, 256), dtype=np.float32)      # rbc head selection
    for p in range(2):
        for j in range(128):
            sel[32 * (2 * p + j // 64), p * 128 + j] = 1.0
    sel = sel.astype(bf)
    in_maps = []
    for g in range(G):
        in_maps.append({
            "xT": xT,
            "wqT": np.ascontiguousarray(Wq[g * GS * HD:(g + 1) * GS * HD, :].T).astype(bf),
            "wkvT": np.ascontiguousarray(
                np.concatenate([Wk[g * HD:(g + 1) * HD, :],
                                Wv[g * HD:(g + 1) * HD, :]], axis=0).T).astype(bf),
            "woT": np.ascontiguousarray(Wo[:, g * GS * HD:(g + 1) * GS * HD].T).astype(bf),
            "masks": masks,
            "aux": aux,
            "sel": sel,
        })
    return in_maps


def kernel(x, Wq, Wk, Wv, Wo):
    x = np.asarray(x, dtype=np.float32)
    in_maps = prep_inputs(np.asarray(x, np.float32), np.asarray(Wq, np.float32),
                          np.asarray(Wk, np.float32), np.asarray(Wv, np.float32),
                          np.asarray(Wo, np.float32))
    nc = build_nc()
    res = run_bass_kernel_spmd(nc, in_maps, list(range(G)))
    acc = np.zeros((T, D), dtype=np.float64)
    for g in range(G):
        acc += res.results[g]["outp"].astype(np.float64)
    return acc.astype(np.float32).reshape(B, S, D)
